# revision 42
# baseline (speedup 1.0000x reference)
"""Trainium2 Bass kernel for 12-head MHA (B=8, S=1024, D=768) over 8 NeuronCores.

Sharding: data-parallel over batch (the hint's M=8 with all 12 heads per
device) - core c computes batch element c entirely, weights replicated.
Returns (h, scores) like the reference.

Per-core dataflow (S=1024 tokens, D=768, H=12 heads, Dh=64):
  1. Stream x and Wq/Wk/Wv into SBUF, PE-transpose to get xT [din, tok] and
     WT [din, dout] (contraction dim must be the partition dim for matmul).
  2. Projections on PE (float32r matmuls, 1 cycle/row):
       qT/kT [dout, tok]  (lhsT = WT chunk, rhs = xT)
       v     [tok, dout]  (lhsT = xT chunk, rhs = WvT), stored bf16 with a
       ones-column appended per head ([tok, 65] per head).
     Biases are folded in as rank-1 matmuls into the same PSUM accumulation.
  3. Per head: scoresT = K Q^T on PE -> exp via ScalarE (scale=1/sqrt(Dh)
     fused, bf16 out, unnormalized).  A V directly from the transposed layout:
     hT[65, q] = v_aug^T . expT; row 64 is the softmax denominator (ones col).
  4. hT is PE-transposed back to [q, 65]; DVE computes recip of col 64 and
     scales cols 0..63 into the output row tile.
  5. The probs output [q, k] is produced by PE-transposing the bf16 exp tiles
     (1 cycle/row) into PSUM and scaling by recip on DVE during PSUM->SBUF.
"""

import sys

if "/opt/trn_rl_repo" not in sys.path:
    sys.path.insert(0, "/opt/trn_rl_repo")

from contextlib import ExitStack

import numpy as np

from concourse import bacc, mybir, tile
from concourse.bass_utils import run_bass_kernel_spmd
from concourse.masks import make_identity

P = 128
S = 1024
D = 768
H = 12
DH = 64
NT = S // P  # 8 token chunks
ND = D // P  # 6 d-model chunks
NCORES = 8
SCALE = 1.0 / np.sqrt(DH)

F32 = mybir.dt.float32
F32R = mybir.dt.float32r
BF16 = mybir.dt.bfloat16
Exp = mybir.ActivationFunctionType.Exp


def _r(ap):
    """View an fp32 AP as float32r so the PE runs the fast fp32 mode."""
    return ap.bitcast(F32R)


def _mha_body(tc, x, W, b, scores, hout):
    nc = tc.nc
    with ExitStack() as ctx:
        const = ctx.enter_context(tc.tile_pool(name="const", bufs=1))
        ident_f = const.tile([P, P], F32, tag="idf", name="idf")
        make_identity(nc, ident_f[:])
        ident_b = const.tile([P, P], BF16, tag="idb", name="idb")
        make_identity(nc, ident_b[:])
        ones_f = const.tile([1, S], F32, tag="ones_f", name="ones_f")
        nc.gpsimd.memset(ones_f[:], 1.0)
        ones = const.tile([1, S], F32R, tag="ones", name="ones")
        nc.vector.tensor_copy(ones[:], ones_f[:])

        # Tiles that live for (almost) the whole kernel.
        persist = ctx.enter_context(tc.tile_pool(name="persist", bufs=1))
        qT = [persist.tile([P, S], F32R, tag=f"qT{i}", name=f"qT{i}") for i in range(ND)]
        kT = [persist.tile([P, S], F32R, tag=f"kT{i}", name=f"kT{i}") for i in range(ND)]
        v_aug = [
            persist.tile([P, H * (DH + 1)], BF16, tag=f"v{t}", name=f"v{t}")
            for t in range(NT)
        ]
        h_row = [
            persist.tile([P, D], F32, tag=f"hr{t}", name=f"hr{t}") for t in range(NT)
        ]

        # ---- setup scope: transposes of x and W, then the projections ----
        with ExitStack() as setup:
            spool = setup.enter_context(tc.tile_pool(name="stream", bufs=3))
            xTp = setup.enter_context(tc.tile_pool(name="xTp", bufs=1))
            wTp = setup.enter_context(tc.tile_pool(name="wTp", bufs=1))
            tps = setup.enter_context(tc.tile_pool(name="tps", bufs=4, space="PSUM"))
            pps = setup.enter_context(tc.tile_pool(name="pps", bufs=2, space="PSUM"))

            xT = [xTp.tile([P, S], F32R, tag=f"xT{i}", name=f"xT{i}") for i in range(ND)]
            WT = {
                nm: [
                    wTp.tile([P, D], F32R, tag=f"WT{nm}{i}", name=f"WT{nm}{i}")
                    for i in range(ND)
                ]
                for nm in ("q", "k", "v")
            }

            # x [tok, din] -> xT [din, tok]
            for t in range(NT):
                xt = spool.tile([P, D], F32, tag="ld", name="ld")
                nc.sync.dma_start(xt[:], x[t * P : (t + 1) * P, :])
                for dc in range(ND):
                    ps = tps.tile([P, P], F32, tag="tp", name="tp")
                    nc.tensor.transpose(ps[:], xt[:, dc * P : (dc + 1) * P], ident_f[:])
                    nc.vector.tensor_copy(xT[dc][:, t * P : (t + 1) * P], ps[:])

            # W [dout, din] -> WT [din, dout]
            for nm in ("q", "k", "v"):
                for ro in range(ND):
                    wt = spool.tile([P, D], F32, tag="ld", name="ld")
                    nc.sync.dma_start(wt[:], W[nm][ro * P : (ro + 1) * P, :])
                    for dc in range(ND):
                        ps = tps.tile([P, P], F32, tag="tp", name="tp")
                        nc.tensor.transpose(
                            ps[:], wt[:, dc * P : (dc + 1) * P], ident_f[:]
                        )
                        if dc % 2 == 0:
                            nc.vector.tensor_copy(
                                WT[nm][dc][:, ro * P : (ro + 1) * P], ps[:]
                            )
                        else:
                            nc.scalar.copy(WT[nm][dc][:, ro * P : (ro + 1) * P], ps[:])

            bias_sb = {}
            bias_pc = {}
            for nm in ("q", "k", "v"):
                t = const.tile([1, D], F32, tag=f"b_{nm}", name=f"b_{nm}")
                nc.sync.dma_start(t[:], b[nm][:])
                tr_ = const.tile([1, D], F32R, tag=f"br_{nm}", name=f"br_{nm}")
                nc.vector.tensor_copy(tr_[:], t[:])
                bias_sb[nm] = tr_
                pc = const.tile([P, ND], F32, tag=f"bp_{nm}", name=f"bp_{nm}")
                nc.sync.dma_start(
                    pc[:], b[nm].rearrange("o (c p) -> p (o c)", p=P)
                )
                bias_pc[nm] = pc

            # q/k projections
            for oc in range(ND):
                for nm, dstT in (("q", qT), ("k", kT)):
                    ps = pps.tile([P, S], F32, tag="proj", name="proj")
                    for half in range(2):
                        lo = half * 512
                        for dc in range(ND):
                            nc.tensor.matmul(
                                ps[:, lo : lo + 512],
                                _r(WT[nm][dc][:, oc * P : (oc + 1) * P]),
                                _r(xT[dc][:, lo : lo + 512]),
                                start=(dc == 0),
                                stop=False,
                            )
                        nc.tensor.matmul(
                            ps[:, lo : lo + 512],
                            _r(bias_sb[nm][0:1, oc * P : (oc + 1) * P]),
                            _r(ones[0:1, lo : lo + 512]),
                            start=False,
                            stop=True,
                        )
                    nc.vector.tensor_copy(dstT[oc][:], ps[:])

            # v projection: v[tok, dout] = x @ Wv.T + bv, stored bf16 + ones col
            for t in range(NT):
                ps = pps.tile([P, S], F32, tag="proj", name="proj")
                for half, n in ((0, 512), (1, 256)):
                    lo = half * 512
                    for dc in range(ND):
                        nc.tensor.matmul(
                            ps[:, lo : lo + n],
                            _r(xT[dc][:, t * P : (t + 1) * P]),
                            _r(WT["v"][dc][:, lo : lo + n]),
                            start=(dc == 0),
                            stop=False,
                        )
                    nc.tensor.matmul(
                        ps[:, lo : lo + n],
                        _r(ones[0:1, 0:P]),
                        _r(bias_sb["v"][0:1, lo : lo + n]),
                        start=False,
                        stop=True,
                    )
                nc.gpsimd.memset(v_aug[t][:], 1.0)
                dst = v_aug[t][:].rearrange("p (h e) -> p h e", h=H)[:, :, 0:DH]
                src = ps[:, 0:D].rearrange("p (h e) -> p h e", h=H)
                nc.vector.tensor_copy(dst, src)

        # ---- attention scope ----
        with ExitStack() as att:
            kqp = att.enter_context(tc.tile_pool(name="kqp", bufs=2, space="PSUM"))
            avp = att.enter_context(tc.tile_pool(name="avp", bufs=1, space="PSUM"))
            trp = att.enter_context(tc.tile_pool(name="trp", bufs=2, space="PSUM"))
            expp = att.enter_context(tc.tile_pool(name="expp", bufs=16))
            probsp = att.enter_context(tc.tile_pool(name="probsp", bufs=4))
            htsp = att.enter_context(tc.tile_pool(name="htsp", bufs=2))
            rpool = att.enter_context(tc.tile_pool(name="rpool", bufs=24))

            for h in range(H):
                hc, hr = divmod(h, 2)
                qh = qT[hc][hr * DH : (hr + 1) * DH, :]
                kh = kT[hc][hr * DH : (hr + 1) * DH, :]

                # scoresT = K Q^T / sqrt(Dh), exponentiated (unnormalized, bf16)
                expT = []
                for kc in range(NT):
                    ps = kqp.tile([P, S], F32, tag="kq", name="kq")
                    for half in range(2):
                        lo = half * 512
                        nc.tensor.matmul(
                            ps[:, lo : lo + 512],
                            _r(kh[:, kc * P : (kc + 1) * P]),
                            _r(qh[:, lo : lo + 512]),
                            start=True,
                            stop=True,
                        )
                    et = expp.tile([P, S], BF16, tag="expT", name="expT")
                    nc.scalar.activation(et[:], ps[:], Exp, scale=float(SCALE))
                    expT.append(et)

                # hT[65, q] = v_aug^T . expT ; row 64 = softmax denominator
                hts = htsp.tile([DH + 1, S], F32, tag="hts", name="hts")
                for half in range(2):
                    lo = half * 512
                    av = avp.tile([DH + 1, 512], F32, tag="av", name="av")
                    for kc in range(NT):
                        nc.tensor.matmul(
                            av[:],
                            v_aug[kc][:, h * (DH + 1) : (h + 1) * (DH + 1)],
                            expT[kc][:, lo : lo + 512],
                            start=(kc == 0),
                            stop=(kc == NT - 1),
                        )
                    nc.scalar.copy(hts[:, lo : lo + 512], av[:])

                # transpose hT back to [q, 65]; recip; scale h into h_row
                recips = []
                for qc in range(NT):
                    ps = trp.tile([P, DH + 1], F32, tag="tr65", name="tr65", bufs=1)
                    nc.tensor.transpose(
                        ps[:],
                        hts[:, qc * P : (qc + 1) * P],
                        ident_f[0 : DH + 1, 0 : DH + 1],
                    )
                    rc = rpool.tile([P, 1], F32, tag="recip", name="recip")
                    nc.vector.reciprocal(rc[:], ps[:, DH : DH + 1])
                    nc.vector.tensor_scalar_mul(
                        h_row[qc][:, h * DH : (h + 1) * DH], ps[:, 0:DH], rc[:]
                    )
                    recips.append(rc)

                # probs output: PE-transpose exp tiles, scale by recip on DVE
                for qc in range(NT):
                    pb = probsp.tile([P, S], F32, tag="probs", name="probs")
                    ps = trp.tile([P, S], BF16, tag="trb", name="trb")
                    for kc in range(NT):
                        nc.tensor.transpose(
                            ps[:, kc * P : (kc + 1) * P],
                            expT[kc][:, qc * P : (qc + 1) * P],
                            ident_b[:],
                        )
                    nc.vector.tensor_scalar_mul(pb[:], ps[:], recips[qc][:])
                    nc.sync.dma_start(scores[h, qc * P : (qc + 1) * P, :], pb[:])

            for qc in range(NT):
                nc.sync.dma_start(hout[qc * P : (qc + 1) * P, :], h_row[qc][:])


def build_program():
    nc = bacc.Bacc("TRN2", target_bir_lowering=False, debug=False)
    x = nc.dram_tensor("x", [S, D], F32, kind="ExternalInput").ap()
    W = {}
    b = {}
    for nm in ("q", "k", "v"):
        W[nm] = nc.dram_tensor(f"W{nm}", [D, D], F32, kind="ExternalInput").ap()
        b[nm] = nc.dram_tensor(f"b{nm}", [1, D], F32, kind="ExternalInput").ap()
    scores = nc.dram_tensor("scores", [H, S, S], F32, kind="ExternalOutput").ap()
    hout = nc.dram_tensor("h", [S, D], F32, kind="ExternalOutput").ap()
    with tile.TileContext(nc) as tc:
        _mha_body(tc, x, W, b, scores, hout)
    return nc


_FINALIZED_NC = None


def _get_nc():
    global _FINALIZED_NC
    if _FINALIZED_NC is None:
        nc = build_program()
        nc.finalize()
        _FINALIZED_NC = nc
    return _FINALIZED_NC


def kernel(x, Wq, bq, Wk, bk, Wv, bv):
    x = np.asarray(x, dtype=np.float32)
    in_common = {
        "Wq": np.asarray(Wq, np.float32),
        "bq": np.asarray(bq, np.float32).reshape(1, D),
        "Wk": np.asarray(Wk, np.float32),
        "bk": np.asarray(bk, np.float32).reshape(1, D),
        "Wv": np.asarray(Wv, np.float32),
        "bv": np.asarray(bv, np.float32).reshape(1, D),
    }
    B = x.shape[0]
    assert B == NCORES, f"expected batch {NCORES}, got {B}"
    in_maps = [{"x": np.ascontiguousarray(x[c]), **in_common} for c in range(B)]
    nc = _get_nc()
    res = run_bass_kernel_spmd(nc, in_maps, core_ids=list(range(NCORES)))
    h = np.stack([res.results[c]["h"] for c in range(B)])
    scores = np.stack([res.results[c]["scores"] for c in range(B)])
    return h, scores
